# revision 9
# baseline (speedup 1.0000x reference)
"""Trainium2 Bass kernel for the EnsembleFeatureLoss OT problem.

Math (per ensemble member e of E=4):
  s = l2norm_rows(gts[e]); t = l2norm_rows(feats[e])      # [4096, 1024]
  sim = s @ t.T                                            # [4096, 4096]
  K = exp(10*sim - 10)
  Sinkhorn converges in exactly 2 iterations for this regime (verified
  against the reference with ~1e5x margin on both sides of the 0.01
  threshold; re-verified on the host from kernel outputs, with a full
  numpy fallback if that check ever fails):
    r1 = u / rowsum(K);  c1 = v / (K.T @ r1)
    r2 = u / (K @ c1);   c2 = v / (K.T @ r2)
  loss_e = sum(outer(r2, c2) * K * sim) = c2 . Z,  Z[n] = sum_m r2 K sim

Distribution: 8 cores = 4 members x 2 row-halves (2048 rows each).
Each core runs two fused passes over its [2048, 4096] block:
  pass A: bf16 matmul -> sim' chunks -> (bf16 spill to DRAM) -> exp with
          fused rowsum accum -> r1 -> P1 += K*r1 (fused STT).
  pair AllReduce of Y1 = colsum(P1) -> c1 (16KB collective).
  pass B: reload sim', exp, fused c1-weighted row-dots -> r2, and the
          P2 / PZ accumulators; Y2/Z colsums via PE ones-matmul.
Host combines per-core [4096] partial vectors (O(N) work only) and does
the 4-scalar ensemble weighting.

Normalization trick: operands stay *unnormalized* bf16; 1/|t| is folded
into the tT operand, and 1/|s| rides the per-partition scale AP of the
ACT exp (K = exp(10*inv_s[m]*sim' - 10)) and the r2 scalar of the Z
accumulator. inv-norms use exp(-0.5*ln(x)) (Ln/Exp are ~2ULP) instead of
the loose-tolerance Sqrt table.
"""

import numpy as np
import ml_dtypes

BF16 = ml_dtypes.bfloat16

E = 4
M = 4096
N = 4096
D = 1024
P = 128
NCORES = 8
MHALF = M // 2              # rows per core
CH = 512                    # psum chunk (one fp32 bank)

_CACHE = {}


def build_bass(mhalf=MHALF, n=N, d=D, ncores=NCORES, m_total=None):
    import concourse.bass as bass
    import concourse.mybir as mybir
    import concourse.tile as tile
    from concourse import bacc
    from concourse.bass import ts

    dt = mybir.dt
    f32, bf16 = dt.float32, dt.bfloat16
    Alu = mybir.AluOpType
    Act = mybir.ActivationFunctionType

    if m_total is None:
        m_total = 2 * mhalf
    nt_m = mhalf // P
    nd = d // P
    nch = n // CH
    n_s_ch = mhalf // CH
    u32 = float(np.float32(1.0 / m_total))
    v32 = float(np.float32(1.0 / n))
    rg = [[i, i + 1] for i in range(0, ncores, 2)]

    nc = bacc.Bacc("TRN2", target_bir_lowering=False, debug=False,
                   num_devices=ncores)
    sT = nc.declare_dram_parameter("sT", [d, mhalf], bf16, isOutput=False)
    tT = nc.declare_dram_parameter("tT", [d, n], bf16, isOutput=False)
    vecs = nc.declare_dram_parameter("vecs", [2, n], f32, isOutput=True)
    r1o = nc.declare_dram_parameter("r1o", [P, nt_m], f32, isOutput=True)
    r2o = nc.declare_dram_parameter("r2o", [P, nt_m], f32, isOutput=True)

    with tile.TileContext(nc) as tc:
        with (
            tc.tile_pool(name="persist", bufs=1) as pp,
            tc.tile_pool(name="opt", bufs=8) as optp,     # tT blocks / pass-B big tiles
            tc.tile_pool(name="ops", bufs=8) as opsp,     # sT blocks
            tc.tile_pool(name="prol", bufs=3) as prolp,   # squares / invt_bc / pass-A K
            tc.tile_pool(name="cst", bufs=3) as cstp,     # psum->bf16 chunk staging
            tc.tile_pool(name="vec", bufs=1) as vecp,     # [1,N]-ish fp32 vectors
            tc.tile_pool(name="vech", bufs=1) as vechp,   # [1,N] bf16 vectors
            tc.tile_pool(name="sm", bufs=8) as smp,       # tiny per-tile stats
            tc.tile_pool(name="ps", bufs=8, space="PSUM") as psp,
            tc.tile_pool(name="dram", bufs=1, space="DRAM") as dp,
        ):
            # ---- dram scratch ----
            simd = dp.tile([mhalf, n], bf16, name="simd", tag="simd")
            y1_in = dp.tile([1, n], f32, name="y1_in", tag="y1_in")
            y1_out = dp.tile([1, n], f32, name="y1_out", tag="y1_out")
            invt_d = dp.tile([1, n], bf16, name="invt_d", tag="invt_d")
            invs_d = dp.tile([1, mhalf], f32, name="invs_d", tag="invs_d")
            c1_d = dp.tile([1, n], bf16, name="c1_d", tag="c1_d")

            # ---- persistent sbuf ----
            tTb = [optp.tile([P, n], bf16, name=f"tTb{b}", tag="opt")
                   for b in range(nd)]
            sTb = [opsp.tile([P, mhalf], bf16, name=f"sTb{b}", tag="ops")
                   for b in range(nd)]
            c1_bc = pp.tile([P, n], bf16, name="c1_bc", tag="c1_bc")
            P1 = pp.tile([P, n], bf16, name="P1", tag="P1")
            P2 = pp.tile([P, n], bf16, name="P2", tag="P2")
            PZ = pp.tile([P, n], bf16, name="PZ", tag="PZ")
            ones = pp.tile([P, 1], bf16, name="ones", tag="ones")
            inv_s = pp.tile([P, nt_m], f32, name="inv_s", tag="inv_s")
            scale10 = pp.tile([P, nt_m], f32, name="scale10", tag="scale10")
            r1buf = pp.tile([P, nt_m], f32, name="r1buf", tag="r1buf")
            r2buf = pp.tile([P, nt_m], f32, name="r2buf", tag="r2buf")
            biasm10 = pp.tile([P, 1], f32, name="biasm10", tag="biasm10")

            nc.vector.memset(biasm10[:], -10.0)
            nc.vector.memset(ones[:], 1.0)
            nc.vector.memset(P1[:], 0.0)
            nc.vector.memset(P2[:], 0.0)
            nc.vector.memset(PZ[:], 0.0)

            # ---- input loads ----
            for b in range(nd):
                nc.sync.dma_start(tTb[b][:], tT[ts(b, P), :])
            for b in range(nd):
                nc.sync.dma_start(sTb[b][:], sT[ts(b, P), :])

            # ---- t-norms: norm2 = colsum(t*t) via square + PE ones-matmul
            pn_t = [psp.tile([1, CH], f32, name=f"pnt{c}", tag="ps")
                    for c in range(nch)]
            for b in range(nd):
                sq = prolp.tile([P, n], bf16, name="sq", tag="prol")
                nc.scalar.square(sq[:], tTb[b][:])
                for c in range(nch):
                    nc.tensor.matmul(pn_t[c][:], ones[:], sq[:, ts(c, CH)],
                                     start=(b == 0), stop=(b == nd - 1))
            # inv = exp(-0.5*ln(norm2))
            normt = vecp.tile([1, n], f32, name="normt", tag="vec")
            for c in range(nch):
                nc.scalar.activation(normt[0:1, ts(c, CH)], pn_t[c][:], Act.Ln)
            invt_h = vechp.tile([1, n], bf16, name="invt_h", tag="vech")
            nc.scalar.activation(invt_h[0:1, :], normt[0:1, :], Act.Exp,
                                 scale=-0.5)
            nc.sync.dma_start(invt_d[:], invt_h[0:1, :])
            invt_bc = prolp.tile([P, n], bf16, name="invt_bc", tag="prol")
            nc.sync.dma_start(invt_bc[:], invt_d[0:1, :].to_broadcast((P, n)))
            # fold 1/|t| into the tT operand
            for b in range(nd):
                nc.vector.tensor_mul(tTb[b][:], tTb[b][:], invt_bc[:])

            # ---- s-norms ----
            pn_s = [psp.tile([1, CH], f32, name=f"pns{c}", tag="ps")
                    for c in range(n_s_ch)]
            for b in range(nd):
                sqs = prolp.tile([P, mhalf], bf16, name="sqs", tag="prol")
                nc.vector.tensor_mul(sqs[:], sTb[b][:], sTb[b][:])
                for c in range(n_s_ch):
                    nc.tensor.matmul(pn_s[c][:], ones[:], sqs[:, ts(c, CH)],
                                     start=(b == 0), stop=(b == nd - 1))
            norms = vecp.tile([1, mhalf], f32, name="norms", tag="vec")
            for c in range(n_s_ch):
                nc.scalar.activation(norms[0:1, ts(c, CH)], pn_s[c][:], Act.Ln)
            nc.scalar.activation(norms[0:1, :], norms[0:1, :], Act.Exp,
                                 scale=-0.5)
            nc.sync.dma_start(invs_d[:], norms[0:1, :])
            # reshape [1, mhalf] -> [P, nt_m]: row m = mi*128 + p -> [p, mi]
            nc.sync.dma_start(
                inv_s[:],
                invs_d[0:1, :].rearrange("a (m p) -> (a p) m", p=P))
            nc.vector.tensor_scalar_mul(scale10[:], inv_s[:], 10.0)

            # ---- pass A ----
            for mi in range(nt_m):
                K = prolp.tile([P, n], bf16, name="K", tag="prol")
                rs8 = smp.tile([P, nch], f32, name="rs8", tag="sm")
                for ni in range(nch):
                    pm = psp.tile([P, CH], f32, name="pm", tag="ps")
                    for dd in range(nd):
                        nc.tensor.matmul(
                            pm[:],
                            sTb[dd][:, ts(mi, P)],
                            tTb[dd][:, ts(ni, CH)],
                            start=(dd == 0), stop=(dd == nd - 1))
                    cst = cstp.tile([P, CH], bf16, name="cst", tag="cst")
                    nc.vector.tensor_copy(cst[:], pm[:])
                    nc.sync.dma_start(simd[ts(mi, P), ts(ni, CH)], cst[:])
                    nc.scalar.activation(K[:, ts(ni, CH)], cst[:], Act.Exp,
                                         bias=biasm10[:],
                                         scale=scale10[:, mi:mi + 1],
                                         accum_out=rs8[:, ni:ni + 1])
                rowsum = smp.tile([P, 1], f32, name="rowsum", tag="sm")
                nc.vector.tensor_reduce(rowsum[:], rs8[:],
                                        mybir.AxisListType.X, Alu.add)
                rinv = smp.tile([P, 1], f32, name="rinv", tag="sm")
                nc.vector.reciprocal(rinv[:], rowsum[:])
                nc.vector.tensor_scalar_mul(r1buf[:, mi:mi + 1], rinv[:], u32)
                nc.vector.scalar_tensor_tensor(
                    out=P1[:], in0=K[:], scalar=r1buf[:, mi:mi + 1],
                    in1=P1[:], op0=Alu.mult, op1=Alu.add)

            # ---- Y1 = colsum(P1); pair AllReduce; c1 ----
            y1sb = vecp.tile([1, n], f32, name="y1sb", tag="vec")
            for c in range(nch):
                py = psp.tile([1, CH], f32, name="py", tag="ps")
                nc.tensor.matmul(py[:], ones[:], P1[:, ts(c, CH)],
                                 start=True, stop=True)
                nc.scalar.copy(y1sb[0:1, ts(c, CH)], py[:])
            nc.sync.dma_start(y1_in[:], y1sb[0:1, :])
            nc.gpsimd.collective_compute(
                "AllReduce", Alu.add, replica_groups=rg,
                ins=[y1_in.opt()], outs=[y1_out.opt()])
            c1f = vecp.tile([1, n], f32, name="c1f", tag="vec")
            nc.sync.dma_start(c1f[0:1, :], y1_out[:])
            nc.vector.reciprocal(c1f[0:1, :], c1f[0:1, :])
            c1h = vechp.tile([1, n], bf16, name="c1h", tag="vech")
            nc.vector.tensor_scalar_mul(c1h[0:1, :], c1f[0:1, :], v32)
            nc.sync.dma_start(c1_d[:], c1h[0:1, :])
            nc.sync.dma_start(c1_bc[:], c1_d[0:1, :].to_broadcast((P, n)))

            # ---- pass B ----
            for mi in range(nt_m):
                stage = optp.tile([P, n], bf16, name="stage2", tag="opt")
                nc.sync.dma_start(stage[:], simd[ts(mi, P), :])
                K = optp.tile([P, n], bf16, name="K2", tag="opt")
                nc.scalar.activation(K[:], stage[:], Act.Exp,
                                     bias=biasm10[:],
                                     scale=scale10[:, mi:mi + 1])
                junk = optp.tile([P, n], bf16, name="junk", tag="opt")
                rowdot = smp.tile([P, 1], f32, name="rowdot", tag="sm")
                nc.vector.scalar_tensor_tensor(
                    out=junk[:], in0=K[:], scalar=1.0, in1=c1_bc[:],
                    op0=Alu.mult, op1=Alu.mult, accum_out=rowdot[:])
                rdinv = smp.tile([P, 1], f32, name="rdinv", tag="sm")
                nc.vector.reciprocal(rdinv[:], rowdot[:])
                nc.vector.tensor_scalar_mul(r2buf[:, mi:mi + 1], rdinv[:], u32)
                r2a = smp.tile([P, 1], f32, name="r2a", tag="sm")
                nc.vector.tensor_mul(r2a[:], r2buf[:, mi:mi + 1],
                                     inv_s[:, mi:mi + 1])
                nc.vector.scalar_tensor_tensor(
                    out=P2[:], in0=K[:], scalar=r2buf[:, mi:mi + 1],
                    in1=P2[:], op0=Alu.mult, op1=Alu.add)
                ks = optp.tile([P, n], bf16, name="ks", tag="opt")
                nc.vector.tensor_mul(ks[:], K[:], stage[:])
                nc.vector.scalar_tensor_tensor(
                    out=PZ[:], in0=ks[:], scalar=r2a[:],
                    in1=PZ[:], op0=Alu.mult, op1=Alu.add)

            # ---- outputs: Y2 = colsum(P2), Z = colsum(PZ) ----
            y2sb = vecp.tile([1, n], f32, name="y2sb", tag="vec")
            for c in range(nch):
                py2 = psp.tile([1, CH], f32, name="py2", tag="ps")
                nc.tensor.matmul(py2[:], ones[:], P2[:, ts(c, CH)],
                                 start=True, stop=True)
                nc.scalar.copy(y2sb[0:1, ts(c, CH)], py2[:])
            nc.sync.dma_start(vecs[0:1, :], y2sb[0:1, :])
            zsb = vecp.tile([1, n], f32, name="zsb", tag="vec")
            for c in range(nch):
                pz2 = psp.tile([1, CH], f32, name="pz2", tag="ps")
                nc.tensor.matmul(pz2[:], ones[:], PZ[:, ts(c, CH)],
                                 start=True, stop=True)
                nc.scalar.copy(zsb[0:1, ts(c, CH)], pz2[:])
            nc.sync.dma_start(vecs[1:2, :], zsb[0:1, :])
            nc.sync.dma_start(r1o[:, :], r1buf[:])
            nc.sync.dma_start(r2o[:, :], r2buf[:])

    return nc


def _make_in_maps(gts, feats):
    in_maps = []
    for core in range(NCORES):
        e, h = divmod(core, 2)
        s_half = gts[e][h * MHALF:(h + 1) * MHALF]          # [2048, 1024]
        in_maps.append({
            "sT": np.ascontiguousarray(s_half.T).astype(BF16),
            "tT": np.ascontiguousarray(feats[e].T).astype(BF16),
        })
    return in_maps


def _ensemble(losses, prev_losses):
    l = np.asarray(losses, np.float64)
    ratio = l / (np.asarray(prev_losses, np.float64) + 1e-8)
    w = np.exp(ratio / 1.0)
    w = w / np.sum(w) * l.shape[0]
    return np.float32(np.sum(w * l))


def _numpy_reference(gts, feats, prev_losses):
    """Faithful float32 fallback, used only if the on-device convergence
    check is violated (never observed for this problem's regime)."""
    losses = []
    for e in range(gts.shape[0]):
        s = gts[e] / np.maximum(
            np.linalg.norm(gts[e], axis=1, keepdims=True), 1e-12)
        t = feats[e] / np.maximum(
            np.linalg.norm(feats[e], axis=1, keepdims=True), 1e-12)
        sim = (s @ t.T).astype(np.float32)
        K = np.exp(-(1.0 - sim) / 0.1)
        m, n = sim.shape
        u = np.full(m, 1.0 / m, np.float32)
        v = np.full(n, 1.0 / n, np.float32)
        r = np.ones(m, np.float32)
        c = np.ones(n, np.float32)
        err = np.inf
        for _ in range(100):
            if err < 0.01:
                break
            r_new = u / (K @ c)
            c = v / (K.T @ r_new)
            err = float(np.mean(np.abs(r_new - r)))
            r = r_new
        losses.append(np.sum(np.outer(r, c) * K * sim))
    return _ensemble(losses, prev_losses)


def _run(gts, feats, trace=False):
    from concourse.bass_utils import run_bass_kernel_spmd
    if "nc" not in _CACHE:
        nc = build_bass()
        nc.finalize()
        _CACHE["nc"] = nc
    in_maps = _make_in_maps(gts, feats)
    return run_bass_kernel_spmd(_CACHE["nc"], in_maps,
                                list(range(NCORES)), trace=trace)


def _combine(results, gts, feats, prev_losses):
    losses = []
    ok = True
    for e in range(E):
        a, b = results[2 * e], results[2 * e + 1]
        Y2 = a["vecs"][0].astype(np.float64) + b["vecs"][0].astype(np.float64)
        Z = a["vecs"][1].astype(np.float64) + b["vecs"][1].astype(np.float64)
        c2 = (1.0 / N) / Y2
        losses.append(np.sum(c2 * Z))
        r1 = np.concatenate([a["r1o"].T.reshape(-1), b["r1o"].T.reshape(-1)])
        r2 = np.concatenate([a["r2o"].T.reshape(-1), b["r2o"].T.reshape(-1)])
        err1 = np.mean(np.abs(r1 - 1.0))
        err2 = np.mean(np.abs(r2 - r1))
        if not (err1 >= 0.01 and err2 < 0.01):
            ok = False
    if not ok:
        return _numpy_reference(gts, feats, prev_losses)
    return _ensemble(losses, prev_losses)


def kernel(gts, feats, prev_losses):
    gts = np.asarray(gts, np.float32)
    feats = np.asarray(feats, np.float32)
    prev_losses = np.asarray(prev_losses, np.float32)
    res = _run(gts, feats)
    return _combine(res.results, gts, feats, prev_losses)


# revision 12
# speedup vs baseline: 1.1985x; 1.1985x over previous
"""Trainium2 Bass kernel for the EnsembleFeatureLoss OT problem.

Math (per ensemble member e of E=4):
  s = l2norm_rows(gts[e]); t = l2norm_rows(feats[e])      # [4096, 1024]
  sim = s @ t.T                                            # [4096, 4096]
  K = exp(10*sim - 10)
  Sinkhorn converges in exactly 2 iterations for this regime (verified
  against the reference with ~1e5x margin on both sides of the 0.01
  threshold; re-verified on the host from kernel outputs, with a full
  numpy fallback if that check ever fails):
    r1 = u / rowsum(K);  c1 = v / (K.T @ r1)
    r2 = u / (K @ c1);   c2 = v / (K.T @ r2)
  loss_e = sum(outer(r2, c2) * K * sim) = c2 . Z,  Z[n] = sum_m r2 K sim

Distribution: 8 cores = 4 members x 2 row-halves (2048 rows each).
Each core runs two fused passes over its [2048, 4096] block:
  pass A: bf16 matmul -> sim' chunks -> (bf16 spill to DRAM) -> exp with
          fused rowsum accum -> r1 -> P1 += K*r1 (fused STT).
  pair AllReduce of Y1 = colsum(P1) -> c1 (16KB collective).
  pass B: reload sim', exp, fused c1-weighted row-dots -> r2, and the
          P2 / PZ accumulators; Y2/Z colsums via PE ones-matmul.
Host combines per-core [4096] partial vectors (O(N) work only) and does
the 4-scalar ensemble weighting.

Normalization trick: operands stay *unnormalized* bf16; 1/|t| is folded
into the tT operand, and 1/|s| rides the per-partition scale AP of the
ACT exp (K = exp(10*inv_s[m]*sim' - 10)) and the r2 scalar of the Z
accumulator. inv-norms use exp(-0.5*ln(x)) (Ln/Exp are ~2ULP) instead of
the loose-tolerance Sqrt table.
"""

import numpy as np
import ml_dtypes

BF16 = ml_dtypes.bfloat16

E = 4
M = 4096
N = 4096
D = 1024
P = 128
NCORES = 8
MHALF = M // 2              # rows per core
CH = 512                    # psum chunk (one fp32 bank)

_CACHE = {}


def build_bass(mhalf=MHALF, n=N, d=D, ncores=NCORES, m_total=None):
    import concourse.bass as bass
    import concourse.mybir as mybir
    import concourse.tile as tile
    from concourse import bacc
    from concourse.bass import ts

    dt = mybir.dt
    f32, bf16 = dt.float32, dt.bfloat16
    Alu = mybir.AluOpType
    Act = mybir.ActivationFunctionType

    if m_total is None:
        m_total = 2 * mhalf
    nt_m = mhalf // P
    nd = d // P
    nch = n // CH
    n_s_ch = mhalf // CH
    u32 = float(np.float32(1.0 / m_total))
    v32 = float(np.float32(1.0 / n))
    rg = [[i, i + 1] for i in range(0, ncores, 2)]

    nc = bacc.Bacc("TRN2", target_bir_lowering=False, debug=False,
                   num_devices=ncores)
    sT = nc.declare_dram_parameter("sT", [d, mhalf], bf16, isOutput=False)
    tT = nc.declare_dram_parameter("tT", [d, n], bf16, isOutput=False)
    vecs = nc.declare_dram_parameter("vecs", [2, n], f32, isOutput=True)
    r1o = nc.declare_dram_parameter("r1o", [P, nt_m], f32, isOutput=True)
    r2o = nc.declare_dram_parameter("r2o", [P, nt_m], f32, isOutput=True)

    with tile.TileContext(nc) as tc:
        with (
            tc.tile_pool(name="persist", bufs=1) as pp,
            tc.tile_pool(name="opt", bufs=8) as optp,     # tT blocks / pass-B big tiles
            tc.tile_pool(name="ops", bufs=8) as opsp,     # sT blocks
            tc.tile_pool(name="prol", bufs=3) as prolp,   # squares / invt_bc / pass-A K
            tc.tile_pool(name="stage", bufs=3) as stagep,  # sim bf16 staging tiles
            tc.tile_pool(name="vec", bufs=1) as vecp,     # [1,N]-ish fp32 vectors
            tc.tile_pool(name="vech", bufs=1) as vechp,   # [1,N] bf16 vectors
            tc.tile_pool(name="sm", bufs=8) as smp,       # tiny per-tile stats
            tc.tile_pool(name="ps", bufs=8, space="PSUM") as psp,
            tc.tile_pool(name="dram", bufs=1, space="DRAM") as dp,
        ):
            # ---- dram scratch ----
            simd = dp.tile([mhalf, n], bf16, name="simd", tag="simd")
            y1_in = dp.tile([1, n], f32, name="y1_in", tag="y1_in")
            y1_out = dp.tile([1, n], f32, name="y1_out", tag="y1_out")
            invt_d = dp.tile([1, n], bf16, name="invt_d", tag="invt_d")
            invs_d = dp.tile([1, mhalf], f32, name="invs_d", tag="invs_d")
            c1_d = dp.tile([1, n], bf16, name="c1_d", tag="c1_d")

            # ---- persistent sbuf ----
            tTb = [optp.tile([P, n], bf16, name=f"tTb{b}", tag="opt")
                   for b in range(nd)]
            sTb = [opsp.tile([P, mhalf], bf16, name=f"sTb{b}", tag="ops")
                   for b in range(nd)]
            c1_bc = pp.tile([P, n], bf16, name="c1_bc", tag="c1_bc")
            P1 = pp.tile([P, n], bf16, name="P1", tag="P1")
            ones = pp.tile([P, 1], bf16, name="ones", tag="ones")
            inv_s = pp.tile([P, nt_m], f32, name="inv_s", tag="inv_s")
            scale10 = pp.tile([P, nt_m], f32, name="scale10", tag="scale10")
            r1buf = pp.tile([P, nt_m], f32, name="r1buf", tag="r1buf")
            r2buf = pp.tile([P, nt_m], f32, name="r2buf", tag="r2buf")
            biasm10 = pp.tile([P, 1], f32, name="biasm10", tag="biasm10")

            nc.vector.memset(biasm10[:], -10.0)
            nc.vector.memset(ones[:], 1.0)
            nc.vector.memset(P1[:], 0.0)

            # ---- input loads ----
            for b in range(nd):
                nc.sync.dma_start(tTb[b][:], tT[ts(b, P), :])
            for b in range(nd):
                nc.sync.dma_start(sTb[b][:], sT[ts(b, P), :])

            # ---- t-norms: norm2 = colsum(t*t) via square + PE ones-matmul
            pn_t = [psp.tile([1, CH], f32, name=f"pnt{c}", tag="ps")
                    for c in range(nch)]
            for b in range(nd):
                sq = prolp.tile([P, n], bf16, name="sq", tag="prol")
                nc.scalar.square(sq[:], tTb[b][:])
                for c in range(nch):
                    nc.tensor.matmul(pn_t[c][:], ones[:], sq[:, ts(c, CH)],
                                     start=(b == 0), stop=(b == nd - 1))
            # inv = exp(-0.5*ln(norm2))
            normt = vecp.tile([1, n], f32, name="normt", tag="vec")
            for c in range(nch):
                nc.scalar.activation(normt[0:1, ts(c, CH)], pn_t[c][:], Act.Ln)
            invt_h = vechp.tile([1, n], bf16, name="invt_h", tag="vech")
            nc.scalar.activation(invt_h[0:1, :], normt[0:1, :], Act.Exp,
                                 scale=-0.5)
            nc.sync.dma_start(invt_d[:], invt_h[0:1, :])
            invt_bc = prolp.tile([P, n], bf16, name="invt_bc", tag="prol")
            nc.sync.dma_start(invt_bc[:], invt_d[0:1, :].to_broadcast((P, n)))
            # fold 1/|t| into the tT operand
            for b in range(nd):
                nc.vector.tensor_mul(tTb[b][:], tTb[b][:], invt_bc[:])

            # ---- s-norms ----
            pn_s = [psp.tile([1, CH], f32, name=f"pns{c}", tag="ps")
                    for c in range(n_s_ch)]
            for b in range(nd):
                sqs = prolp.tile([P, mhalf], bf16, name="sqs", tag="prol")
                nc.vector.tensor_mul(sqs[:], sTb[b][:], sTb[b][:])
                for c in range(n_s_ch):
                    nc.tensor.matmul(pn_s[c][:], ones[:], sqs[:, ts(c, CH)],
                                     start=(b == 0), stop=(b == nd - 1))
            norms = vecp.tile([1, mhalf], f32, name="norms", tag="vec")
            for c in range(n_s_ch):
                nc.scalar.activation(norms[0:1, ts(c, CH)], pn_s[c][:], Act.Ln)
            nc.scalar.activation(norms[0:1, :], norms[0:1, :], Act.Exp,
                                 scale=-0.5)
            nc.sync.dma_start(invs_d[:], norms[0:1, :])
            # reshape [1, mhalf] -> [P, nt_m]: row m = mi*128 + p -> [p, mi]
            nc.sync.dma_start(
                inv_s[:],
                invs_d[0:1, :].rearrange("a (m p) -> (a p) m", p=P))
            nc.vector.tensor_scalar_mul(scale10[:], inv_s[:], 10.0)

            # ---- pass A ----
            for mi in range(nt_m):
                stage = stagep.tile([P, n], bf16, name="stage", tag="stage")
                K = prolp.tile([P, n], bf16, name="K", tag="prol")
                rs8 = smp.tile([P, nch], f32, name="rs8", tag="sm")
                for ni in range(nch):
                    pm = psp.tile([P, CH], f32, name="pm", tag="ps")
                    for dd in range(nd):
                        nc.tensor.matmul(
                            pm[:],
                            sTb[dd][:, ts(mi, P)],
                            tTb[dd][:, ts(ni, CH)],
                            start=(dd == 0), stop=(dd == nd - 1))
                    nc.scalar.copy(stage[:, ts(ni, CH)], pm[:])
                    nc.scalar.activation(K[:, ts(ni, CH)], pm[:], Act.Exp,
                                         bias=biasm10[:],
                                         scale=scale10[:, mi:mi + 1],
                                         accum_out=rs8[:, ni:ni + 1])
                nc.sync.dma_start(simd[ts(mi, P), :], stage[:])
                rowsum = smp.tile([P, 1], f32, name="rowsum", tag="sm")
                nc.vector.tensor_reduce(rowsum[:], rs8[:],
                                        mybir.AxisListType.X, Alu.add)
                rinv = smp.tile([P, 1], f32, name="rinv", tag="sm")
                nc.vector.reciprocal(rinv[:], rowsum[:])
                nc.vector.tensor_scalar_mul(r1buf[:, mi:mi + 1], rinv[:], u32)
                nc.vector.scalar_tensor_tensor(
                    out=P1[:], in0=K[:], scalar=r1buf[:, mi:mi + 1],
                    in1=P1[:], op0=Alu.mult, op1=Alu.add)

            # ---- Y1 = colsum(P1); pair AllReduce; c1 ----
            y1sb = vecp.tile([1, n], f32, name="y1sb", tag="vec")
            for c in range(nch):
                py = psp.tile([1, CH], f32, name="py", tag="ps")
                nc.tensor.matmul(py[:], ones[:], P1[:, ts(c, CH)],
                                 start=True, stop=True)
                nc.scalar.copy(y1sb[0:1, ts(c, CH)], py[:])
            nc.sync.dma_start(y1_in[:], y1sb[0:1, :])
            nc.gpsimd.collective_compute(
                "AllReduce", Alu.add, replica_groups=rg,
                ins=[y1_in.opt()], outs=[y1_out.opt()])
            c1f = vecp.tile([1, n], f32, name="c1f", tag="vec")
            nc.sync.dma_start(c1f[0:1, :], y1_out[:])
            nc.vector.reciprocal(c1f[0:1, :], c1f[0:1, :])
            c1h = vechp.tile([1, n], bf16, name="c1h", tag="vech")
            nc.vector.tensor_scalar_mul(c1h[0:1, :], c1f[0:1, :], v32)
            nc.sync.dma_start(c1_d[:], c1h[0:1, :])
            nc.sync.dma_start(c1_bc[:], c1_d[0:1, :].to_broadcast((P, n)))

            # ---- pass B ----
            # Y2/Z accumulate on the (otherwise idle) PE: per column chunk
            # one psum bank holds Y2 at partition 0 and Z at partition 32.
            # Banks are zeroed by DVE and all matmuls run start=False, so
            # first-touch overwrite/accumulate is order- and state-proof.
            pyz = [psp.tile([P, CH], f32, name=f"pyz{c}", tag="ps")
                   for c in range(nch)]
            for c in range(nch):
                nc.vector.memset(pyz[c][:], 0.0)
            for mi in range(nt_m):
                stage = optp.tile([P, n], bf16, name="stage2", tag="opt")
                nc.sync.dma_start(stage[:], simd[ts(mi, P), :])
                K = optp.tile([P, n], bf16, name="K2", tag="opt")
                nc.scalar.activation(K[:], stage[:], Act.Exp,
                                     bias=biasm10[:],
                                     scale=scale10[:, mi:mi + 1])
                kc = optp.tile([P, n], bf16, name="kc", tag="opt")
                nc.vector.tensor_mul(kc[:], K[:], c1_bc[:])
                rowdot = smp.tile([P, 1], f32, name="rowdot", tag="sm")
                nc.vector.tensor_reduce(rowdot[:], kc[:],
                                        mybir.AxisListType.X, Alu.add)
                rdinv = smp.tile([P, 1], f32, name="rdinv", tag="sm")
                nc.vector.reciprocal(rdinv[:], rowdot[:])
                nc.vector.tensor_scalar_mul(r2buf[:, mi:mi + 1], rdinv[:], u32)
                r2h = smp.tile([P, 1], bf16, name="r2h", tag="smh")
                nc.vector.tensor_copy(r2h[:], r2buf[:, mi:mi + 1])
                r2ah = smp.tile([P, 1], bf16, name="r2ah", tag="smh")
                nc.vector.tensor_mul(r2ah[:], r2buf[:, mi:mi + 1],
                                     inv_s[:, mi:mi + 1])
                ks = optp.tile([P, n], bf16, name="ks", tag="opt")
                nc.vector.tensor_mul(ks[:], K[:], stage[:])
                for c in range(nch):
                    nc.tensor.matmul(pyz[c][0:1, :],
                                     r2h[:], K[:, ts(c, CH)],
                                     start=False, stop=(mi == nt_m - 1),
                                     skip_group_check=True)
                    nc.tensor.matmul(pyz[c][32:33, :],
                                     r2ah[:], ks[:, ts(c, CH)],
                                     start=False, stop=(mi == nt_m - 1),
                                     skip_group_check=True)

            # ---- outputs ----
            y2sb = vecp.tile([1, n], f32, name="y2sb", tag="vec")
            zsb = vecp.tile([1, n], f32, name="zsb", tag="vec")
            for c in range(nch):
                nc.scalar.copy(y2sb[0:1, ts(c, CH)], pyz[c][0:1, :])
                nc.scalar.copy(zsb[0:1, ts(c, CH)], pyz[c][32:33, :])
            nc.sync.dma_start(vecs[0:1, :], y2sb[0:1, :])
            nc.sync.dma_start(vecs[1:2, :], zsb[0:1, :])
            nc.sync.dma_start(r1o[:, :], r1buf[:])
            nc.sync.dma_start(r2o[:, :], r2buf[:])

    return nc


def _make_in_maps(gts, feats):
    in_maps = []
    for core in range(NCORES):
        e, h = divmod(core, 2)
        s_half = gts[e][h * MHALF:(h + 1) * MHALF]          # [2048, 1024]
        in_maps.append({
            "sT": np.ascontiguousarray(s_half.T).astype(BF16),
            "tT": np.ascontiguousarray(feats[e].T).astype(BF16),
        })
    return in_maps


def _ensemble(losses, prev_losses):
    l = np.asarray(losses, np.float64)
    ratio = l / (np.asarray(prev_losses, np.float64) + 1e-8)
    w = np.exp(ratio / 1.0)
    w = w / np.sum(w) * l.shape[0]
    return np.float32(np.sum(w * l))


def _numpy_reference(gts, feats, prev_losses):
    """Faithful float32 fallback, used only if the on-device convergence
    check is violated (never observed for this problem's regime)."""
    losses = []
    for e in range(gts.shape[0]):
        s = gts[e] / np.maximum(
            np.linalg.norm(gts[e], axis=1, keepdims=True), 1e-12)
        t = feats[e] / np.maximum(
            np.linalg.norm(feats[e], axis=1, keepdims=True), 1e-12)
        sim = (s @ t.T).astype(np.float32)
        K = np.exp(-(1.0 - sim) / 0.1)
        m, n = sim.shape
        u = np.full(m, 1.0 / m, np.float32)
        v = np.full(n, 1.0 / n, np.float32)
        r = np.ones(m, np.float32)
        c = np.ones(n, np.float32)
        err = np.inf
        for _ in range(100):
            if err < 0.01:
                break
            r_new = u / (K @ c)
            c = v / (K.T @ r_new)
            err = float(np.mean(np.abs(r_new - r)))
            r = r_new
        losses.append(np.sum(np.outer(r, c) * K * sim))
    return _ensemble(losses, prev_losses)


def _run(gts, feats, trace=False):
    from concourse.bass_utils import run_bass_kernel_spmd
    if "nc" not in _CACHE:
        nc = build_bass()
        nc.finalize()
        _CACHE["nc"] = nc
    in_maps = _make_in_maps(gts, feats)
    return run_bass_kernel_spmd(_CACHE["nc"], in_maps,
                                list(range(NCORES)), trace=trace)


def _combine(results, gts, feats, prev_losses):
    losses = []
    ok = True
    for e in range(E):
        a, b = results[2 * e], results[2 * e + 1]
        Y2 = a["vecs"][0].astype(np.float64) + b["vecs"][0].astype(np.float64)
        Z = a["vecs"][1].astype(np.float64) + b["vecs"][1].astype(np.float64)
        c2 = (1.0 / N) / Y2
        losses.append(np.sum(c2 * Z))
        r1 = np.concatenate([a["r1o"].T.reshape(-1), b["r1o"].T.reshape(-1)])
        r2 = np.concatenate([a["r2o"].T.reshape(-1), b["r2o"].T.reshape(-1)])
        err1 = np.mean(np.abs(r1 - 1.0))
        err2 = np.mean(np.abs(r2 - r1))
        if not (err1 >= 0.01 and err2 < 0.01):
            ok = False
    if not ok:
        return _numpy_reference(gts, feats, prev_losses)
    return _ensemble(losses, prev_losses)


def kernel(gts, feats, prev_losses):
    gts = np.asarray(gts, np.float32)
    feats = np.asarray(feats, np.float32)
    prev_losses = np.asarray(prev_losses, np.float32)
    res = _run(gts, feats)
    return _combine(res.results, gts, feats, prev_losses)
